# revision 4
# baseline (speedup 1.0000x reference)
"""CPPN dense-MLP Trainium2 kernel.

Network (per point): 3 -> 16 (tanh) -> 8 x [16 -> 16 (tanh)] -> 1 (sigmoid).
2,097,152 points, pure data parallel across 8 NeuronCores.

Per-core layout: the core's 262,144 points are split into S=8 streams of
32,768 points.  Activations live in SBUF/PSUM "block layout": partition
16*j + m holds feature m of stream j, free dim indexes points within the
stream.  Every layer is then a single 128x128 block-diagonal stationary
matmul on the tensor engine (8 independent 16x16 matmuls per cycle).

Layer 0 (K=3) needs x transposed (features on partitions); instead of any
transpose we DMA x contiguously as [8 streams, 3*cols] and run three
accumulating matmuls, one per input feature, whose rhs is a stride-3 AP
into that buffer and whose [8, 128] stationary scatters W0[:, f] into the
block-diagonal positions.

Activations (tanh / sigmoid + bias) run on the scalar engine directly from
PSUM into SBUF; tanh and sigmoid share one ACT table set so there are no
table reloads.

Matmuls are full float32 (4 cycles/column): this network doubles any
injected error every layer (~x250 over the 9-layer chain), so float32r's
~2^-13 per-product noise lands at ~0.3 absolute output error while fp32
stays at ~3e-5.  fp32 is mandatory for the gate, making the kernel
PE-bound.
"""

import numpy as np
import ml_dtypes
from contextlib import ExitStack


def round_f32r(a):
    """Round fp32 to the float32r grid (value representable as bf16 + bf16)."""
    a = np.asarray(a, np.float32)
    hi = a.astype(ml_dtypes.bfloat16).astype(np.float32)
    lo = (a - hi).astype(ml_dtypes.bfloat16).astype(np.float32)
    return hi + lo

import concourse.bass as bass
import concourse.tile as tile
from concourse import bacc, mybir
from concourse.bass_utils import run_bass_kernel_spmd

F32 = mybir.dt.float32
F32R = mybir.dt.float32r

N_FULL = 2097152
N_CORES = 8
N_CORE = N_FULL // N_CORES  # 262144 points per core
S = 8                       # streams per core
W = 16                      # hidden width
N_HIDDEN = 8


def format_inputs(W0, b0, Wh, bh, Wo, bo):
    """Build the block-diagonal stationary matrices + bias table (numpy)."""
    W0 = np.asarray(W0, np.float32)
    b0 = np.asarray(b0, np.float32)
    Wh = np.asarray(Wh, np.float32)
    bh = np.asarray(bh, np.float32)
    Wo = np.asarray(Wo, np.float32)
    bo = np.asarray(bo, np.float32)

    # Layer 0: three [S, 128] stationaries (one per input feature), packed
    # side by side into [S, 3*128].  stationary_f[j, 16j+m] = W0[m, f].
    w0f = np.zeros((S, 3 * 128), np.float32)
    for f in range(3):
        for j in range(S):
            w0f[j, f * 128 + 16 * j:f * 128 + 16 * j + W] = W0[:, f]

    # Hidden layers: [128, 8*128]; slice l is blockdiag(Wh[l].T x8):
    # stat[16j+fi, 16j+m] = Wh[l][m, fi].
    wh = np.zeros((128, N_HIDDEN * 128), np.float32)
    for l in range(N_HIDDEN):
        for j in range(S):
            r = 16 * j
            wh[r:r + W, l * 128 + r:l * 128 + r + W] = Wh[l].T

    # Output layer: [128, S]: stat[16j+fi, j] = Wo[0, fi].
    wo = np.zeros((128, S), np.float32)
    for j in range(S):
        wo[16 * j:16 * j + W, j] = Wo[0, :]

    # Bias table [128, 10]: col 0 = b0 block, cols 1..8 = bh blocks,
    # col 9 rows 0..7 = bo.
    bias = np.zeros((128, 10), np.float32)
    for j in range(S):
        bias[16 * j:16 * j + W, 0] = b0
        for l in range(N_HIDDEN):
            bias[16 * j:16 * j + W, 1 + l] = bh[l]
    bias[0:S, 9] = bo[0]

    return {"w0f": w0f, "wh": wh, "wo": wo, "bias": bias}


def build_program(n_core=N_CORE, g_cols=1024, f_cols=512, repeat=1,
                  num_devices=N_CORES):
    """Build + compile the per-core Bass program (SPMD: same on all cores).

    n_core: points per core.  g_cols: free-dim columns per group (pipeline
    granularity; one ACT instruction per layer per group).  f_cols: free-dim
    columns per matmul (<= 512, one PSUM bank).  repeat: run the whole
    kernel body this many times (for wall-clock timing; output idempotent).
    """
    stream_len = n_core // S
    ng = stream_len // g_cols
    assert stream_len % g_cols == 0 and g_cols % f_cols == 0
    cpg = g_cols // f_cols  # matmul chunks per group

    nc = bacc.Bacc("TRN2", target_bir_lowering=False, debug=False,
                   num_devices=num_devices)
    x_ap = nc.dram_tensor("x", [n_core, 3], F32, kind="ExternalInput").ap()
    w0f_ap = nc.dram_tensor("w0f", [S, 3 * 128], F32, kind="ExternalInput").ap()
    wh_ap = nc.dram_tensor("wh", [128, N_HIDDEN * 128], F32,
                           kind="ExternalInput").ap()
    wo_ap = nc.dram_tensor("wo", [128, S], F32, kind="ExternalInput").ap()
    bias_ap = nc.dram_tensor("bias", [128, 10], F32, kind="ExternalInput").ap()
    y_ap = nc.dram_tensor("y", [n_core, 1], F32, kind="ExternalOutput").ap()

    # DRAM views: stream-major.  x[(j n) f] -> [S, stream_len*3] so each
    # partition's group slice is one contiguous run.
    xr = x_ap.rearrange("(j n) f -> j (n f)", j=S)
    yr = y_ap.rearrange("(j n) o -> j (n o)", j=S)

    Tanh = mybir.ActivationFunctionType.Tanh
    Sigmoid = mybir.ActivationFunctionType.Sigmoid

    with tile.TileContext(nc) as tc, ExitStack() as ctx:
        consts = ctx.enter_context(tc.tile_pool(name="consts", bufs=1))
        xpool = ctx.enter_context(tc.tile_pool(name="xpool", bufs=3))
        hpool = ctx.enter_context(tc.tile_pool(name="hpool", bufs=3))
        ypool = ctx.enter_context(tc.tile_pool(name="ypool", bufs=3))
        zpool = ctx.enter_context(
            tc.tile_pool(name="zpool", bufs=4, space="PSUM"))

        w0f_sb = consts.tile([S, 3 * 128], F32)
        nc.sync.dma_start(w0f_sb[:], w0f_ap[:])
        wh_sb = consts.tile([128, N_HIDDEN * 128], F32)
        nc.sync.dma_start(wh_sb[:], wh_ap[:])
        wo_sb = consts.tile([128, S], F32)
        nc.sync.dma_start(wo_sb[:], wo_ap[:])
        bias_sb = consts.tile([128, 10], F32)
        nc.sync.dma_start(bias_sb[:], bias_ap[:])

        for _rep in range(repeat):
            for g in range(ng):
                # ---- load x chunk: [S, 3*g_cols], contiguous per stream
                xs = xpool.tile([S, 3 * g_cols], F32, tag="xs")
                nc.sync.dma_start(
                    xs[:], xr[:, g * 3 * g_cols:(g + 1) * 3 * g_cols])
                # stride-3 feature view: [S, g_cols, 3]
                xs3 = xs.rearrange("p (n f) -> p n f", f=3)

                # ---- layer 0: 3 accumulating matmuls per f_cols chunk
                z = zpool.tile([128, g_cols], F32, tag="z")
                for c in range(cpg):
                    for f in range(3):
                        nc.tensor.matmul(
                            z[:, c * f_cols:(c + 1) * f_cols],
                            lhsT=w0f_sb[:, f * 128:(f + 1) * 128],
                            rhs=xs3[:, c * f_cols:(c + 1) * f_cols, f],
                            start=(f == 0), stop=(f == 2),
                        )
                h = hpool.tile([128, g_cols], F32, tag="h")
                nc.scalar.activation(h[:], z[:], Tanh, bias=bias_sb[:, 0:1])

                # ---- hidden layers
                for l in range(N_HIDDEN):
                    z = zpool.tile([128, g_cols], F32, tag="z")
                    for c in range(cpg):
                        nc.tensor.matmul(
                            z[:, c * f_cols:(c + 1) * f_cols],
                            lhsT=wh_sb[:, l * 128:(l + 1) * 128],
                            rhs=h[:, c * f_cols:(c + 1) * f_cols],
                            start=True, stop=True,
                        )
                    h2 = hpool.tile([128, g_cols], F32, tag="h")
                    nc.scalar.activation(h2[:], z[:], Tanh,
                                         bias=bias_sb[:, l + 1:l + 2])
                    h = h2

                # ---- output layer -> [S, g_cols]
                zo = zpool.tile([S, g_cols], F32, tag="z")
                for c in range(cpg):
                    nc.tensor.matmul(
                        zo[:, c * f_cols:(c + 1) * f_cols],
                        lhsT=wo_sb[:],
                        rhs=h[:, c * f_cols:(c + 1) * f_cols],
                        start=True, stop=True,
                    )
                ys = ypool.tile([S, g_cols], F32, tag="ys")
                nc.scalar.activation(ys[:], zo[:], Sigmoid,
                                     bias=bias_sb[0:S, 9:10])
                nc.sync.dma_start(yr[:, g * g_cols:(g + 1) * g_cols], ys[:])

    nc.compile()
    return nc


_PROGRAM_CACHE = {}


def _get_program(key=(N_CORE, 1024, 512, 1)):
    if key not in _PROGRAM_CACHE:
        n_core, g_cols, f_cols, repeat = key
        _PROGRAM_CACHE[key] = build_program(n_core, g_cols, f_cols, repeat)
    return _PROGRAM_CACHE[key]


def kernel(x, W0, b0, Wh, bh, Wo, bo):
    x = np.ascontiguousarray(np.asarray(x, np.float32))
    assert x.shape == (N_FULL, 3), x.shape
    consts = format_inputs(W0, b0, Wh, bh, Wo, bo)

    nc = _get_program()
    in_maps = []
    for c in range(N_CORES):
        m = {"x": x[c * N_CORE:(c + 1) * N_CORE]}
        m.update(consts)
        in_maps.append(m)
    res = run_bass_kernel_spmd(nc, in_maps, core_ids=list(range(N_CORES)),
                               trace=False)
    y = np.concatenate([res.results[c]["y"] for c in range(N_CORES)], axis=0)
    return y.astype(np.float32, copy=False)


# revision 17
# speedup vs baseline: 1.6153x; 1.6153x over previous
"""CPPN dense-MLP Trainium2 kernel.

Network (per point): 3 -> 16 (tanh) -> 8 x [16 -> 16 (tanh)] -> 1 (sigmoid).
2,097,152 points, pure data parallel across 8 NeuronCores.

Per-core layout: the core's 262,144 points are split into S=8 streams of
32,768 points.  Activations live in SBUF/PSUM "block layout": partition
16*j + m holds feature m of stream j, free dim indexes points within the
stream.  Every layer is then a single 128x128 block-diagonal stationary
matmul on the tensor engine (8 independent 16x16 matmuls per cycle).

Layer 0 (K=3) needs x with features on partitions; the host pre-transposes
each core's shard to [24, 32768] (partition 8f+j = feature f of stream j;
a cheap numpy reshape, part of sharding), so layer 0 is a single K=24
matmul with fully contiguous DMA loads.  (Alternative modes kept for A/B:
"three_mm" = stride-3 rhs APs over natural-layout x, ~+110us PE;
"strided" = DMA-side transpose, ~+600us of 4-byte-element descriptors.)

Activations (tanh / sigmoid + bias) run on the scalar engine directly from
PSUM into SBUF; tanh and sigmoid share one ACT table set so there are no
table reloads.

Matmuls are full float32 (4 cycles/column): this network doubles any
injected error every layer (~x250 over the 9-layer chain), so float32r's
~2^-13 per-product noise lands at ~0.3 absolute output error while fp32
stays at ~3e-5.  fp32 is mandatory for the gate, making the kernel
PE-bound.
"""

import numpy as np
import ml_dtypes
from contextlib import ExitStack


def round_f32r(a):
    """Round fp32 to the float32r grid (value representable as bf16 + bf16)."""
    a = np.asarray(a, np.float32)
    hi = a.astype(ml_dtypes.bfloat16).astype(np.float32)
    lo = (a - hi).astype(ml_dtypes.bfloat16).astype(np.float32)
    return hi + lo

import concourse.bass as bass
import concourse.tile as tile
from concourse import bacc, mybir
from concourse.bass_utils import run_bass_kernel_spmd

F32 = mybir.dt.float32
F32R = mybir.dt.float32r

N_FULL = 2097152
N_CORES = 8
N_CORE = N_FULL // N_CORES  # 262144 points per core
S = 8                       # streams per core
W = 16                      # hidden width
N_HIDDEN = 8


def format_inputs(W0, b0, Wh, bh, Wo, bo):
    """Build the block-diagonal stationary matrices + bias table (numpy)."""
    W0 = np.asarray(W0, np.float32)
    b0 = np.asarray(b0, np.float32)
    Wh = np.asarray(Wh, np.float32)
    bh = np.asarray(bh, np.float32)
    Wo = np.asarray(Wo, np.float32)
    bo = np.asarray(bo, np.float32)

    # Layer 0: three [S, 128] stationaries (one per input feature), packed
    # side by side into [S, 3*128].  stationary_f[j, 16j+m] = W0[m, f].
    w0f = np.zeros((S, 3 * 128), np.float32)
    for f in range(3):
        for j in range(S):
            w0f[j, f * 128 + 16 * j:f * 128 + 16 * j + W] = W0[:, f]

    # Hidden layers: [128, 8*128]; slice l is blockdiag(Wh[l].T x8):
    # stat[16j+fi, 16j+m] = Wh[l][m, fi].
    wh = np.zeros((128, N_HIDDEN * 128), np.float32)
    for l in range(N_HIDDEN):
        for j in range(S):
            r = 16 * j
            wh[r:r + W, l * 128 + r:l * 128 + r + W] = Wh[l].T

    # Output layer: [128, S]: stat[16j+fi, j] = Wo[0, fi].
    wo = np.zeros((128, S), np.float32)
    for j in range(S):
        wo[16 * j:16 * j + W, j] = Wo[0, :]

    # Bias table [128, 10]: col 0 = b0 block, cols 1..8 = bh blocks,
    # col 9 rows 0..7 = bo.
    bias = np.zeros((128, 10), np.float32)
    for j in range(S):
        bias[16 * j:16 * j + W, 0] = b0
        for l in range(N_HIDDEN):
            bias[16 * j:16 * j + W, 1 + l] = bh[l]
    bias[0:S, 9] = bo[0]

    # Layer 0 alternative: single [24, 128] stationary for K=24 matmul
    # over a DMA-transposed x (f-major partitions): w0t[8f+j, 16j+m] = W0[m, f].
    w0t = np.zeros((24, 128), np.float32)
    for j in range(S):
        for f in range(3):
            w0t[8 * f + j, 16 * j:16 * j + W] = W0[:, f]

    return {"w0f": w0f, "w0t": w0t, "wh": wh, "wo": wo, "bias": bias}


def build_program(n_core=N_CORE, g_cols=1024, f_cols=512, repeat=1,
                  num_devices=N_CORES, depth=2, xbufs=None, hbufs=None,
                  ybufs=None, zbufs=None, l0_mode="three_mm"):
    """Build + compile the per-core Bass program (SPMD: same on all cores).

    n_core: points per core.  g_cols: free-dim columns per group (pipeline
    granularity; one ACT instruction per layer per group).  f_cols: free-dim
    columns per matmul (<= 512, one PSUM bank).  repeat: run the whole
    kernel body this many times (for wall-clock timing; output idempotent).
    """
    stream_len = n_core // S
    ng = stream_len // g_cols
    assert stream_len % g_cols == 0 and g_cols % f_cols == 0
    cpg = g_cols // f_cols  # matmul chunks per group

    nc = bacc.Bacc("TRN2", target_bir_lowering=False, debug=False,
                   num_devices=num_devices)
    if l0_mode == "host_t":
        xt_ap = nc.dram_tensor("xt", [24, n_core // S], F32,
                               kind="ExternalInput").ap()
        x_ap = None
    else:
        x_ap = nc.dram_tensor("x", [n_core, 3], F32,
                              kind="ExternalInput").ap()
    w0f_ap = nc.dram_tensor("w0f", [S, 3 * 128], F32, kind="ExternalInput").ap()
    w0t_ap = nc.dram_tensor("w0t", [24, 128], F32, kind="ExternalInput").ap()
    wh_ap = nc.dram_tensor("wh", [128, N_HIDDEN * 128], F32,
                           kind="ExternalInput").ap()
    wo_ap = nc.dram_tensor("wo", [128, S], F32, kind="ExternalInput").ap()
    bias_ap = nc.dram_tensor("bias", [128, 10], F32, kind="ExternalInput").ap()
    y_ap = nc.dram_tensor("y", [n_core, 1], F32, kind="ExternalOutput").ap()

    # DRAM views: stream-major.  x[(j n) f] -> [S, stream_len*3] so each
    # partition's group slice is one contiguous run.
    if x_ap is not None:
        xr = x_ap.rearrange("(j n) f -> j (n f)", j=S)
        # transposed view for strided-DMA layer 0: [S, 3, stream_len]
        xt_dram = x_ap.rearrange("(j n) f -> j f n", j=S)
    yr = y_ap.rearrange("(j n) o -> j (n o)", j=S)

    Tanh = mybir.ActivationFunctionType.Tanh
    Sigmoid = mybir.ActivationFunctionType.Sigmoid

    with tile.TileContext(nc) as tc, ExitStack() as ctx:
        if zbufs is None:
            # slots are bank (2KB) granular; fill the 8 banks
            banks_per_slot = max(1, (g_cols * 4) // 2048)
            zbufs = max(2, 8 // banks_per_slot)
        if xbufs is None:
            xbufs = depth + 1
        if hbufs is None:
            hbufs = 2 * depth + 1
        if ybufs is None:
            ybufs = depth + 1
        consts = ctx.enter_context(tc.tile_pool(name="consts", bufs=1))
        xpool = ctx.enter_context(tc.tile_pool(name="xpool", bufs=xbufs))
        hpool = ctx.enter_context(tc.tile_pool(name="hpool", bufs=hbufs))
        ypool = ctx.enter_context(tc.tile_pool(name="ypool", bufs=ybufs))
        zpool = ctx.enter_context(
            tc.tile_pool(name="zpool", bufs=zbufs, space="PSUM"))

        w0f_sb = consts.tile([S, 3 * 128], F32)
        nc.sync.dma_start(w0f_sb[:], w0f_ap[:])
        w0t_sb = consts.tile([24, 128], F32)
        nc.sync.dma_start(w0t_sb[:], w0t_ap[:])
        wh_sb = consts.tile([128, N_HIDDEN * 128], F32)
        nc.sync.dma_start(wh_sb[:], wh_ap[:])
        wo_sb = consts.tile([128, S], F32)
        nc.sync.dma_start(wo_sb[:], wo_ap[:])
        bias_sb = consts.tile([128, 10], F32)
        nc.sync.dma_start(bias_sb[:], bias_ap[:])

        # Interleave `depth` groups at each layer step: within a group the
        # PE's layer l+1 strictly follows ACT of layer l, so a single group
        # serializes PE<->ACT.  Emitting layer l for D groups back-to-back
        # gives the PE work while ACT drains the other groups' PSUM tiles.
        for _rep in range(repeat):
            for g0 in range(0, ng, depth):
                gs = range(g0, min(g0 + depth, ng))
                xss, hs, zs = {}, {}, {}
                for g in gs:
                    if l0_mode == "three_mm":
                        # ---- load x chunk [S, 3*g_cols], contiguous/stream
                        xs = xpool.tile([S, 3 * g_cols], F32, tag="xs")
                        nc.sync.dma_start(
                            xs[:], xr[:, g * 3 * g_cols:(g + 1) * 3 * g_cols])
                        # stride-3 feature view: [S, g_cols, 3]
                        xss[g] = xs.rearrange("p (n f) -> p n f", f=3)
                    elif l0_mode == "host_t":
                        # ---- x pre-transposed on host: contiguous [24, g]
                        xs = xpool.tile([24, g_cols], F32, tag="xs")
                        nc.sync.dma_start(
                            xs[:], xt_ap[:, g * g_cols:(g + 1) * g_cols])
                        xss[g] = xs
                    else:
                        # ---- strided-DMA transpose load: [24, g_cols]
                        xs = xpool.tile([24, g_cols], F32, tag="xs")
                        for f in range(3):
                            nc.sync.dma_start(
                                xs[8 * f:8 * (f + 1), :],
                                xt_dram[:, f, g * g_cols:(g + 1) * g_cols])
                        xss[g] = xs

                # ---- layer 0
                for g in gs:
                    z = zpool.tile([128, g_cols], F32, tag="z")
                    zs[g] = z
                    for c in range(cpg):
                        if l0_mode == "three_mm":
                            for f in range(3):
                                nc.tensor.matmul(
                                    z[:, c * f_cols:(c + 1) * f_cols],
                                    lhsT=w0f_sb[:, f * 128:(f + 1) * 128],
                                    rhs=xss[g][:, c * f_cols:(c + 1) * f_cols,
                                               f],
                                    start=(f == 0), stop=(f == 2),
                                )
                        else:  # host_t / strided: single K=24 matmul
                            nc.tensor.matmul(
                                z[:, c * f_cols:(c + 1) * f_cols],
                                lhsT=w0t_sb[:],
                                rhs=xss[g][:, c * f_cols:(c + 1) * f_cols],
                                start=True, stop=True,
                            )
                for g in gs:
                    h = hpool.tile([128, g_cols], F32, tag="h")
                    nc.scalar.activation(h[:], zs[g][:], Tanh,
                                         bias=bias_sb[:, 0:1])
                    hs[g] = h

                # ---- hidden layers
                for l in range(N_HIDDEN):
                    for g in gs:
                        z = zpool.tile([128, g_cols], F32, tag="z")
                        zs[g] = z
                        for c in range(cpg):
                            nc.tensor.matmul(
                                z[:, c * f_cols:(c + 1) * f_cols],
                                lhsT=wh_sb[:, l * 128:(l + 1) * 128],
                                rhs=hs[g][:, c * f_cols:(c + 1) * f_cols],
                                start=True, stop=True,
                            )
                    for g in gs:
                        h2 = hpool.tile([128, g_cols], F32, tag="h")
                        nc.scalar.activation(h2[:], zs[g][:], Tanh,
                                             bias=bias_sb[:, l + 1:l + 2])
                        hs[g] = h2

                # ---- output layer -> [S, g_cols]
                for g in gs:
                    zo = zpool.tile([S, g_cols], F32, tag="z")
                    zs[g] = zo
                    for c in range(cpg):
                        nc.tensor.matmul(
                            zo[:, c * f_cols:(c + 1) * f_cols],
                            lhsT=wo_sb[:],
                            rhs=hs[g][:, c * f_cols:(c + 1) * f_cols],
                            start=True, stop=True,
                        )
                for g in gs:
                    ys = ypool.tile([S, g_cols], F32, tag="ys")
                    nc.scalar.activation(ys[:], zs[g][:], Sigmoid,
                                         bias=bias_sb[0:S, 9:10])
                    nc.sync.dma_start(yr[:, g * g_cols:(g + 1) * g_cols],
                                      ys[:])

    nc.compile()
    return nc


_RUNNER_CACHE = {}
L0_MODE = "host_t"


def host_transpose_x(x_core):
    """[n_core, 3] -> [24, n_core/S]: partition 8f+j = feature f of stream j."""
    L = x_core.shape[0] // S
    return np.ascontiguousarray(
        x_core.reshape(S, L, 3).transpose(2, 0, 1).reshape(24, L))


def make_in_maps(x, consts, l0_mode=None):
    """Per-core input maps from the full x [N_FULL, 3] + formatted weights."""
    l0_mode = l0_mode or L0_MODE
    in_maps = []
    for c in range(N_CORES):
        xc = x[c * N_CORE:(c + 1) * N_CORE]
        if l0_mode == "host_t":
            m = {"xt": host_transpose_x(xc)}
        else:
            m = {"x": np.ascontiguousarray(xc)}
        m.update(consts)
        in_maps.append(m)
    return in_maps


def make_runner(nc, n_cores=N_CORES):
    """Build a reusable jitted PJRT runner for the SPMD program (mirrors
    bass2jax.run_bass_via_pjrt's multi-core path, minus output donation, so
    the NEFF compile is paid once and later calls are just execution)."""
    import jax
    from jax.sharding import Mesh, PartitionSpec, NamedSharding
    from jax.experimental.shard_map import shard_map
    from concourse import bass2jax

    bass2jax.install_neuronx_cc_hook()
    partition_name = (nc.partition_id_tensor.name
                      if nc.partition_id_tensor else None)
    in_names, out_names, out_avals = [], [], []
    for alloc in nc.m.functions[0].allocations:
        if not isinstance(alloc, mybir.MemoryLocationSet):
            continue
        name = alloc.memorylocations[0].name
        if alloc.kind == "ExternalInput":
            if name != partition_name:
                in_names.append(name)
        elif alloc.kind == "ExternalOutput":
            out_names.append(name)
            out_avals.append(jax.core.ShapedArray(
                tuple(alloc.tensor_shape), mybir.dt.np(alloc.dtype)))
    n_params = len(in_names)
    all_in_names = list(in_names) + list(out_names)
    if partition_name is not None:
        all_in_names.append(partition_name)

    def _body(*args):
        operands = list(args)
        if partition_name is not None:
            operands.append(bass2jax.partition_id_tensor())
        outs = bass2jax._bass_exec_p.bind(
            *operands,
            out_avals=tuple(out_avals),
            in_names=tuple(all_in_names),
            out_names=tuple(out_names),
            lowering_input_output_aliases=(),
            sim_require_finite=True,
            sim_require_nnan=True,
            nc=nc,
        )
        return tuple(outs)

    devices = jax.devices()[:n_cores]
    mesh = Mesh(np.asarray(devices), ("core",))
    n_outs = len(out_names)
    in_specs = (PartitionSpec("core"),) * (n_params + n_outs)
    out_specs = (PartitionSpec("core"),) * n_outs
    fn = jax.jit(shard_map(_body, mesh=mesh, in_specs=in_specs,
                           out_specs=out_specs, check_rep=False),
                 keep_unused=True)
    sharding = NamedSharding(mesh, PartitionSpec("core"))

    def prepare(in_maps):
        concat_in = [
            np.concatenate([np.asarray(in_maps[c][n])
                            for c in range(n_cores)], axis=0)
            for n in in_names
        ]
        concat_zero = [
            np.zeros((n_cores * a.shape[0],) + tuple(a.shape[1:]), a.dtype)
            for a in out_avals
        ]
        return [jax.device_put(a, sharding) for a in concat_in + concat_zero]

    return fn, prepare, out_names


def _get_runner(key=(N_CORE, 1024, 512, 1, L0_MODE)):
    if key not in _RUNNER_CACHE:
        n_core, g_cols, f_cols, repeat, l0_mode = key
        nc = build_program(n_core, g_cols, f_cols, repeat, l0_mode=l0_mode)
        _RUNNER_CACHE[key] = make_runner(nc)
    return _RUNNER_CACHE[key]


def kernel(x, W0, b0, Wh, bh, Wo, bo):
    import jax
    x = np.asarray(x, np.float32)
    assert x.shape == (N_FULL, 3), x.shape
    consts = format_inputs(W0, b0, Wh, bh, Wo, bo)
    fn, prepare, out_names = _get_runner()
    args = prepare(make_in_maps(x, consts))
    outs = fn(*args)
    jax.block_until_ready(outs)
    y = np.asarray(outs[out_names.index("y")])
    return np.ascontiguousarray(y.reshape(N_FULL, 1).astype(np.float32))


# revision 25
# speedup vs baseline: 2.2547x; 1.3958x over previous
"""CPPN dense-MLP Trainium2 kernel.

Network (per point): 3 -> 16 (tanh) -> 8 x [16 -> 16 (tanh)] -> 1 (sigmoid).
2,097,152 points, pure data parallel across 8 NeuronCores.

Per-core layout: the core's 262,144 points are split into S=8 streams of
32,768 points.  Activations live in SBUF/PSUM "block layout": partition
16*j + m holds feature m of stream j, free dim indexes points within the
stream.  Every layer is then a single 128x128 block-diagonal stationary
matmul on the tensor engine (8 independent 16x16 matmuls per cycle).

Layer 0 (K=3) needs x with features on partitions; the host pre-transposes
each core's shard to [24, 32768] (partition 8f+j = feature f of stream j;
a cheap numpy reshape, part of sharding), so layer 0 is a single K=24
matmul with fully contiguous DMA loads.  (Alternative modes kept for A/B:
"three_mm" = stride-3 rhs APs over natural-layout x, ~+110us PE;
"strided" = DMA-side transpose, ~+600us of 4-byte-element descriptors.)

Activations (tanh / sigmoid + bias) run on the scalar engine directly from
PSUM into SBUF; tanh and sigmoid share one ACT table set so there are no
table reloads.

Matmuls are full float32 (4 cycles/column): this network doubles any
injected error every layer (~x250 over the 9-layer chain), so float32r's
~2^-13 per-product noise lands at ~0.3 absolute output error while fp32
stays at ~3e-5.  fp32 is mandatory for the gate, making the kernel
PE-bound.
"""

import numpy as np
import ml_dtypes
from contextlib import ExitStack


def round_f32r(a):
    """Round fp32 to the float32r grid (value representable as bf16 + bf16)."""
    a = np.asarray(a, np.float32)
    hi = a.astype(ml_dtypes.bfloat16).astype(np.float32)
    lo = (a - hi).astype(ml_dtypes.bfloat16).astype(np.float32)
    return hi + lo

import concourse.bass as bass
import concourse.tile as tile
from concourse import bacc, mybir
from concourse.bass_utils import run_bass_kernel_spmd

F32 = mybir.dt.float32
F32R = mybir.dt.float32r

N_FULL = 2097152
N_CORES = 8
N_CORE = N_FULL // N_CORES  # 262144 points per core
S = 8                       # streams per core
W = 16                      # hidden width
N_HIDDEN = 8


def format_inputs(W0, b0, Wh, bh, Wo, bo):
    """Build the block-diagonal stationary matrices + bias table (numpy)."""
    W0 = np.asarray(W0, np.float32)
    b0 = np.asarray(b0, np.float32)
    Wh = np.asarray(Wh, np.float32)
    bh = np.asarray(bh, np.float32)
    Wo = np.asarray(Wo, np.float32)
    bo = np.asarray(bo, np.float32)

    # Layer 0: three [S, 128] stationaries (one per input feature), packed
    # side by side into [S, 3*128].  stationary_f[j, 16j+m] = W0[m, f].
    w0f = np.zeros((S, 3 * 128), np.float32)
    for f in range(3):
        for j in range(S):
            w0f[j, f * 128 + 16 * j:f * 128 + 16 * j + W] = W0[:, f]

    # Hidden layers: [128, 8*128]; slice l is blockdiag(Wh[l].T x8):
    # stat[16j+fi, 16j+m] = Wh[l][m, fi].
    wh = np.zeros((128, N_HIDDEN * 128), np.float32)
    for l in range(N_HIDDEN):
        for j in range(S):
            r = 16 * j
            wh[r:r + W, l * 128 + r:l * 128 + r + W] = Wh[l].T

    # Output layer: [128, S]: stat[16j+fi, j] = Wo[0, fi].
    wo = np.zeros((128, S), np.float32)
    for j in range(S):
        wo[16 * j:16 * j + W, j] = Wo[0, :]

    # Bias table [128, 10]: col 0 = b0 block, cols 1..8 = bh blocks,
    # col 9 rows 0..7 = bo.
    bias = np.zeros((128, 10), np.float32)
    for j in range(S):
        bias[16 * j:16 * j + W, 0] = b0
        for l in range(N_HIDDEN):
            bias[16 * j:16 * j + W, 1 + l] = bh[l]
    bias[0:S, 9] = bo[0]

    # Layer 0 alternative: single [24, 128] stationary for K=24 matmul
    # over a DMA-transposed x (f-major partitions): w0t[8f+j, 16j+m] = W0[m, f].
    w0t = np.zeros((24, 128), np.float32)
    for j in range(S):
        for f in range(3):
            w0t[8 * f + j, 16 * j:16 * j + W] = W0[:, f]

    return {"w0f": w0f, "w0t": w0t, "wh": wh, "wo": wo, "bias": bias}


def build_program(n_core=N_CORE, g_cols=1024, f_cols=512, repeat=1,
                  num_devices=N_CORES, depth=2, xbufs=None, hbufs=None,
                  ybufs=None, zbufs=None, l0_mode="three_mm"):
    """Build + compile the per-core Bass program (SPMD: same on all cores).

    n_core: points per core.  g_cols: free-dim columns per group (pipeline
    granularity; one ACT instruction per layer per group).  f_cols: free-dim
    columns per matmul (<= 512, one PSUM bank).  repeat: run the whole
    kernel body this many times (for wall-clock timing; output idempotent).
    """
    stream_len = n_core // S
    ng = stream_len // g_cols
    assert stream_len % g_cols == 0 and g_cols % f_cols == 0
    cpg = g_cols // f_cols  # matmul chunks per group

    nc = bacc.Bacc("TRN2", target_bir_lowering=False, debug=False,
                   num_devices=num_devices)
    if l0_mode == "host_t":
        xt_ap = nc.dram_tensor("xt", [24, n_core // S], F32,
                               kind="ExternalInput").ap()
        x_ap = None
    else:
        x_ap = nc.dram_tensor("x", [n_core, 3], F32,
                              kind="ExternalInput").ap()
    w0f_ap = nc.dram_tensor("w0f", [S, 3 * 128], F32, kind="ExternalInput").ap()
    w0t_ap = nc.dram_tensor("w0t", [24, 128], F32, kind="ExternalInput").ap()
    wh_ap = nc.dram_tensor("wh", [128, N_HIDDEN * 128], F32,
                           kind="ExternalInput").ap()
    wo_ap = nc.dram_tensor("wo", [128, S], F32, kind="ExternalInput").ap()
    bias_ap = nc.dram_tensor("bias", [128, 10], F32, kind="ExternalInput").ap()
    y_ap = nc.dram_tensor("y", [n_core, 1], F32, kind="ExternalOutput").ap()

    # DRAM views: stream-major.  x[(j n) f] -> [S, stream_len*3] so each
    # partition's group slice is one contiguous run.
    if x_ap is not None:
        xr = x_ap.rearrange("(j n) f -> j (n f)", j=S)
        # transposed view for strided-DMA layer 0: [S, 3, stream_len]
        xt_dram = x_ap.rearrange("(j n) f -> j f n", j=S)
    yr = y_ap.rearrange("(j n) o -> j (n o)", j=S)

    Tanh = mybir.ActivationFunctionType.Tanh
    Sigmoid = mybir.ActivationFunctionType.Sigmoid

    with tile.TileContext(nc) as tc, ExitStack() as ctx:
        if zbufs is None:
            # slots are bank (2KB) granular; fill the 8 banks
            banks_per_slot = max(1, (g_cols * 4) // 2048)
            zbufs = max(2, 8 // banks_per_slot)
        if xbufs is None:
            xbufs = depth + 1
        if hbufs is None:
            hbufs = 2 * depth + 1
        if ybufs is None:
            ybufs = depth + 1
        consts = ctx.enter_context(tc.tile_pool(name="consts", bufs=1))
        xpool = ctx.enter_context(tc.tile_pool(name="xpool", bufs=xbufs))
        hpool = ctx.enter_context(tc.tile_pool(name="hpool", bufs=hbufs))
        ypool = ctx.enter_context(tc.tile_pool(name="ypool", bufs=ybufs))
        zpool = ctx.enter_context(
            tc.tile_pool(name="zpool", bufs=zbufs, space="PSUM"))

        w0f_sb = consts.tile([S, 3 * 128], F32)
        nc.sync.dma_start(w0f_sb[:], w0f_ap[:])
        w0t_sb = consts.tile([24, 128], F32)
        nc.sync.dma_start(w0t_sb[:], w0t_ap[:])
        wh_sb = consts.tile([128, N_HIDDEN * 128], F32)
        nc.sync.dma_start(wh_sb[:], wh_ap[:])
        wo_sb = consts.tile([128, S], F32)
        nc.sync.dma_start(wo_sb[:], wo_ap[:])
        bias_sb = consts.tile([128, 10], F32)
        nc.sync.dma_start(bias_sb[:], bias_ap[:])

        # Interleave `depth` groups at each layer step: within a group the
        # PE's layer l+1 strictly follows ACT of layer l, so a single group
        # serializes PE<->ACT.  Emitting layer l for D groups back-to-back
        # gives the PE work while ACT drains the other groups' PSUM tiles.
        for _rep in range(repeat):
            for g0 in range(0, ng, depth):
                gs = range(g0, min(g0 + depth, ng))
                xss, hs, zs = {}, {}, {}
                for g in gs:
                    if l0_mode == "three_mm":
                        # ---- load x chunk [S, 3*g_cols], contiguous/stream
                        xs = xpool.tile([S, 3 * g_cols], F32, tag="xs")
                        nc.sync.dma_start(
                            xs[:], xr[:, g * 3 * g_cols:(g + 1) * 3 * g_cols])
                        # stride-3 feature view: [S, g_cols, 3]
                        xss[g] = xs.rearrange("p (n f) -> p n f", f=3)
                    elif l0_mode == "host_t":
                        # ---- x pre-transposed on host: contiguous [24, g]
                        xs = xpool.tile([24, g_cols], F32, tag="xs")
                        nc.sync.dma_start(
                            xs[:], xt_ap[:, g * g_cols:(g + 1) * g_cols])
                        xss[g] = xs
                    else:
                        # ---- strided-DMA transpose load: [24, g_cols]
                        xs = xpool.tile([24, g_cols], F32, tag="xs")
                        for f in range(3):
                            nc.sync.dma_start(
                                xs[8 * f:8 * (f + 1), :],
                                xt_dram[:, f, g * g_cols:(g + 1) * g_cols])
                        xss[g] = xs

                # ---- layer 0
                for g in gs:
                    z = zpool.tile([128, g_cols], F32, tag="z")
                    zs[g] = z
                    for c in range(cpg):
                        if l0_mode == "three_mm":
                            for f in range(3):
                                nc.tensor.matmul(
                                    z[:, c * f_cols:(c + 1) * f_cols],
                                    lhsT=w0f_sb[:, f * 128:(f + 1) * 128],
                                    rhs=xss[g][:, c * f_cols:(c + 1) * f_cols,
                                               f],
                                    start=(f == 0), stop=(f == 2),
                                )
                        else:  # host_t / strided: single K=24 matmul
                            nc.tensor.matmul(
                                z[:, c * f_cols:(c + 1) * f_cols],
                                lhsT=w0t_sb[:],
                                rhs=xss[g][:, c * f_cols:(c + 1) * f_cols],
                                start=True, stop=True,
                            )
                for g in gs:
                    h = hpool.tile([128, g_cols], F32, tag="h")
                    nc.scalar.activation(h[:], zs[g][:], Tanh,
                                         bias=bias_sb[:, 0:1])
                    hs[g] = h

                # ---- hidden layers
                for l in range(N_HIDDEN):
                    for g in gs:
                        z = zpool.tile([128, g_cols], F32, tag="z")
                        zs[g] = z
                        for c in range(cpg):
                            nc.tensor.matmul(
                                z[:, c * f_cols:(c + 1) * f_cols],
                                lhsT=wh_sb[:, l * 128:(l + 1) * 128],
                                rhs=hs[g][:, c * f_cols:(c + 1) * f_cols],
                                start=True, stop=True,
                            )
                    for g in gs:
                        h2 = hpool.tile([128, g_cols], F32, tag="h")
                        nc.scalar.activation(h2[:], zs[g][:], Tanh,
                                             bias=bias_sb[:, l + 1:l + 2])
                        hs[g] = h2

                # ---- output layer -> [S, g_cols]
                for g in gs:
                    zo = zpool.tile([S, g_cols], F32, tag="z")
                    zs[g] = zo
                    for c in range(cpg):
                        nc.tensor.matmul(
                            zo[:, c * f_cols:(c + 1) * f_cols],
                            lhsT=wo_sb[:],
                            rhs=hs[g][:, c * f_cols:(c + 1) * f_cols],
                            start=True, stop=True,
                        )
                for g in gs:
                    ys = ypool.tile([S, g_cols], F32, tag="ys")
                    nc.scalar.activation(ys[:], zs[g][:], Sigmoid,
                                         bias=bias_sb[0:S, 9:10])
                    nc.sync.dma_start(yr[:, g * g_cols:(g + 1) * g_cols],
                                      ys[:])

    nc.compile()
    return nc


# ---------------------------------------------------------------------------
# Tile-position variant: 32 streams in 16 pairs, each layer = one "wave" of
# 16 CONCURRENT 32x32 tile matmuls (measured ~3.4x the full-array fp32 rate,
# since a 32x32 tile holding blockdiag(W.T x2) does 50% useful MACs vs the
# 128x128 block-diagonal's 12.5%).  Pair p lives at coordinates
# (strip s, free-block fb) of the current [128, 4F] activation tile; the MM
# for a pair is tile_position (32*s, 32*s') with s' = (s+fb) % 4, writing
# PSUM (strip s', free-block s) -- so positions evolve by the invertible map
# (s, fb) -> ((s+fb)%4, s) and all 16 (row, col) tiles are used exactly once
# per wave.  PE drops to ~1 us per 8192-point wave; the scalar engine's
# tanh throughput becomes the bottleneck.
# ---------------------------------------------------------------------------

N_STREAMS = 32
N_PAIRS = 16
L32 = N_CORE // N_STREAMS    # 8192 points per stream


def _pair_positions():
    """pos[layer][p] = (strip, freeblock) for layers 1..10 (post-L0..output)."""
    pos = [{p: (p // 4, p % 4) for p in range(N_PAIRS)}]
    for _ in range(N_HIDDEN + 1):
        nxt = {}
        for p, (s, fb) in pos[-1].items():
            nxt[p] = ((s + fb) % 4, s)
        pos.append(nxt)
    return pos


def format_inputs_tiles(W0, b0, Wh, bh, Wo, bo):
    W0 = np.asarray(W0, np.float32)
    b0 = np.asarray(b0, np.float32)
    Wh = np.asarray(Wh, np.float32)
    bh = np.asarray(bh, np.float32)
    Wo = np.asarray(Wo, np.float32)
    bo = np.asarray(bo, np.float32)

    # L0 stationary [128, 32]: strip c rows 3*sl+f, cols 16*sl+m = W0[m, f]
    w0t32 = np.zeros((128, 32), np.float32)
    for c in range(4):
        for sl in range(2):
            for f in range(3):
                w0t32[32 * c + 3 * sl + f, 16 * sl:16 * sl + W] = W0[:, f]

    # hidden stationaries [128, 32*8]: strip c = blockdiag(Wh[l].T x2)
    wh32 = np.zeros((128, 32 * N_HIDDEN), np.float32)
    for l in range(N_HIDDEN):
        for c in range(4):
            for sl in range(2):
                r = 32 * c + 16 * sl
                wh32[r:r + W, 32 * l + 16 * sl:32 * l + 16 * sl + W] = Wh[l].T

    # output stationary [128, 32]: strip c rows 16*sl+fi, col sl = Wo[0, fi];
    # cols 2..31 zero so the MM writes its full 32-row strip (cost is
    # per-column, so the padding is free and keeps PSUM fully initialized)
    wo32 = np.zeros((128, 32), np.float32)
    for c in range(4):
        for sl in range(2):
            wo32[32 * c + 16 * sl:32 * c + 16 * sl + W, sl] = Wo[0, :]

    # bias table [128, 10]: tanh cols use rows 32c+16sl+m; sigmoid col 9
    # uses rows 32c+sl
    bias32 = np.zeros((128, 10), np.float32)
    for c in range(4):
        for sl in range(2):
            r = 32 * c + 16 * sl
            bias32[r:r + W, 0] = b0
            for l in range(N_HIDDEN):
                bias32[r:r + W, 1 + l] = bh[l]
            bias32[32 * c + sl, 9] = bo[0]

    return {"w0t32": w0t32, "wh32": wh32, "wo32": wo32, "bias32": bias32}


def host_pack_x_tiles(x_core, f_cols=512):
    """[n_core, 3] -> packed [24, l32*4]: per wave w, strip b rows 6b+3sl+f,
    free w*4F + a*F + n = x[(2*(4a+b)+sl)*l32 + w*F + n, f]."""
    l32 = x_core.shape[0] // N_STREAMS
    nw = l32 // f_cols
    # [a, b, sl, w, n, f] -> [b, sl, f, w, a, n]
    xv = np.asarray(x_core, np.float32).reshape(4, 4, 2, nw, f_cols, 3)
    out = xv.transpose(1, 2, 5, 3, 0, 4)
    return np.ascontiguousarray(out).reshape(24, nw * 4 * f_cols)


def host_unpack_y_tiles(y_raw, f_cols=512):
    """y_raw [128, l32*4] -> y [n_core, 1] using final pair positions."""
    l32 = y_raw.shape[1] // 4
    nw = l32 // f_cols
    pos_out = _pair_positions()[N_HIDDEN + 1]
    yv = y_raw.reshape(8, nw, 4, f_cols)
    rows = np.array([2 * pos_out[p][0] + sl
                     for p in range(N_PAIRS) for sl in range(2)])
    fbs = np.array([pos_out[p][1] for p in range(N_PAIRS)]).repeat(2)
    y = yv[rows, :, fbs, :]          # [32, nw, f_cols]
    return np.ascontiguousarray(y).reshape(N_STREAMS * l32, 1)


def build_program_tiles(n_core=N_CORE, f_cols=512, repeat=1,
                        num_devices=N_CORES, depth=2, xbufs=None, hbufs=None,
                        ybufs=None):
    """Tile-position wave kernel.  One wave = 16 concurrent 32x32 MMs
    (f_cols columns each) into a [128, 4*f_cols] PSUM tile (2 banks at
    f_cols=256), one ACT pass per wave-layer.  `depth` wave-chains are
    interleaved so ACT stays busy while the other chain's MMs run."""
    l32 = n_core // N_STREAMS
    nw = l32 // f_cols
    F = f_cols
    pos = _pair_positions()

    nc = bacc.Bacc("TRN2", target_bir_lowering=False, debug=False,
                   num_devices=num_devices)
    xt_ap = nc.dram_tensor("xt", [24, l32 * 4], F32, kind="ExternalInput").ap()
    w0_ap = nc.dram_tensor("w0t32", [128, 32], F32, kind="ExternalInput").ap()
    wh_ap = nc.dram_tensor("wh32", [128, 32 * N_HIDDEN], F32,
                           kind="ExternalInput").ap()
    wo_ap = nc.dram_tensor("wo32", [128, 32], F32, kind="ExternalInput").ap()
    bias_ap = nc.dram_tensor("bias32", [128, 10], F32,
                             kind="ExternalInput").ap()
    # only rows 32c+sl of the sigmoid output carry data; ship them packed
    y_ap = nc.dram_tensor("y_raw", [8, l32 * 4], F32,
                          kind="ExternalOutput").ap()

    Tanh = mybir.ActivationFunctionType.Tanh
    Sigmoid = mybir.ActivationFunctionType.Sigmoid

    with tile.TileContext(nc) as tc, ExitStack() as ctx:
        consts = ctx.enter_context(tc.tile_pool(name="consts", bufs=1))
        xpool = ctx.enter_context(tc.tile_pool(
            name="xpool", bufs=xbufs or depth + 1))
        hpool = ctx.enter_context(tc.tile_pool(
            name="hpool", bufs=hbufs or 2 * depth + 1))
        ypool = ctx.enter_context(tc.tile_pool(
            name="ypool", bufs=ybufs or depth + 1))
        # one PSUM slot is [128, 4F] with free-block i = one full bank at
        # F=512, so concurrent tile drains never share a (bank, partition)
        zbufs = max(2, 8 // max(1, (4 * f_cols * 4) // 2048))
        zpool = ctx.enter_context(tc.tile_pool(name="zpool", bufs=zbufs,
                                               space="PSUM"))

        w0_sb = consts.tile([128, 32], F32)
        nc.sync.dma_start(w0_sb[:], w0_ap[:])
        wh_sb = consts.tile([128, 32 * N_HIDDEN], F32)
        nc.sync.dma_start(wh_sb[:], wh_ap[:])
        wo_sb = consts.tile([128, 32], F32)
        nc.sync.dma_start(wo_sb[:], wo_ap[:])
        bias_sb = consts.tile([128, 10], F32)
        nc.sync.dma_start(bias_sb[:], bias_ap[:])

        for _rep in range(repeat):
            for w0i in range(0, nw, depth):
                ws = range(w0i, min(w0i + depth, nw))
                xss, hs, zs = {}, {}, {}
                for wv in ws:
                    xs = xpool.tile([128, 4 * F], F32, tag="xs")
                    for b in range(4):
                        nc.sync.dma_start(
                            xs[32 * b:32 * b + 6, :],
                            xt_ap[6 * b:6 * b + 6,
                                  wv * 4 * F:(wv + 1) * 4 * F])
                    xss[wv] = xs

                # ---- layer 0 wave: pair (a,b): tile (b, a), rhs strip b
                # fb a, out (strip a, fb b)
                for wv in ws:
                    z = zpool.tile([128, 4 * F], F32, tag="z")
                    zs[wv] = z
                    for p in range(N_PAIRS):
                        a, b = p // 4, p % 4
                        nc.tensor.matmul(
                            z[32 * a:32 * (a + 1), b * F:(b + 1) * F],
                            lhsT=w0_sb[32 * b:32 * b + 6, :],
                            rhs=xss[wv][32 * b:32 * b + 6,
                                        a * F:(a + 1) * F],
                            start=True, stop=True,
                            tile_position=(32 * b, 32 * a),
                            skip_group_check=True,
                        )
                for wv in ws:
                    h = hpool.tile([128, 4 * F], F32, tag="h")
                    nc.scalar.activation(h[:], zs[wv][:], Tanh,
                                         bias=bias_sb[:, 0:1])
                    hs[wv] = h

                # ---- hidden waves
                for l in range(N_HIDDEN):
                    cur, nxt = pos[l], pos[l + 1]
                    for wv in ws:
                        z = zpool.tile([128, 4 * F], F32, tag="z")
                        zs[wv] = z
                        for p in range(N_PAIRS):
                            s, fb = cur[p]
                            s2 = nxt[p][0]
                            nc.tensor.matmul(
                                z[32 * s2:32 * (s2 + 1), s * F:(s + 1) * F],
                                lhsT=wh_sb[32 * s:32 * (s + 1),
                                           32 * l:32 * (l + 1)],
                                rhs=hs[wv][32 * s:32 * (s + 1),
                                           fb * F:(fb + 1) * F],
                                start=True, stop=True,
                                tile_position=(32 * s, 32 * s2),
                                skip_group_check=True,
                            )
                    for wv in ws:
                        h2 = hpool.tile([128, 4 * F], F32, tag="h")
                        nc.scalar.activation(h2[:], zs[wv][:], Tanh,
                                             bias=bias_sb[:, l + 1:l + 2])
                        hs[wv] = h2

                # ---- output wave: M=2 per tile
                cur, nxt = pos[N_HIDDEN], pos[N_HIDDEN + 1]
                for wv in ws:
                    zo = zpool.tile([128, 4 * F], F32, tag="z")
                    zs[wv] = zo
                    for p in range(N_PAIRS):
                        s, fb = cur[p]
                        s2 = nxt[p][0]
                        nc.tensor.matmul(
                            zo[32 * s2:32 * (s2 + 1), s * F:(s + 1) * F],
                            lhsT=wo_sb[32 * s:32 * (s + 1), :],
                            rhs=hs[wv][32 * s:32 * (s + 1),
                                       fb * F:(fb + 1) * F],
                            start=True, stop=True,
                            tile_position=(32 * s, 32 * s2),
                            skip_group_check=True,
                        )
                for wv in ws:
                    ys = ypool.tile([128, 4 * F], F32, tag="ys")
                    nc.scalar.activation(ys[:], zs[wv][:], Sigmoid,
                                         bias=bias_sb[:, 9:10])
                    for c in range(4):
                        nc.sync.dma_start(
                            y_ap[2 * c:2 * c + 2,
                                 wv * 4 * F:(wv + 1) * 4 * F],
                            ys[32 * c:32 * c + 2, :])

    nc.compile()
    return nc


_RUNNER_CACHE = {}
L0_MODE = "host_t"
KERNEL_MODE = "tiles"   # "waves of 16 concurrent 32x32 tile matmuls" | "block"


def host_transpose_x(x_core):
    """[n_core, 3] -> [24, n_core/S]: partition 8f+j = feature f of stream j."""
    L = x_core.shape[0] // S
    return np.ascontiguousarray(
        x_core.reshape(S, L, 3).transpose(2, 0, 1).reshape(24, L))


def make_in_maps(x, consts, l0_mode=None):
    """Per-core input maps from the full x [N_FULL, 3] + formatted weights."""
    l0_mode = l0_mode or L0_MODE
    in_maps = []
    for c in range(N_CORES):
        xc = x[c * N_CORE:(c + 1) * N_CORE]
        if l0_mode == "host_t":
            m = {"xt": host_transpose_x(xc)}
        else:
            m = {"x": np.ascontiguousarray(xc)}
        m.update(consts)
        in_maps.append(m)
    return in_maps


def make_runner(nc, n_cores=N_CORES):
    """Build a reusable jitted PJRT runner for the SPMD program (mirrors
    bass2jax.run_bass_via_pjrt's multi-core path, minus output donation, so
    the NEFF compile is paid once and later calls are just execution)."""
    import jax
    from jax.sharding import Mesh, PartitionSpec, NamedSharding
    from jax.experimental.shard_map import shard_map
    from concourse import bass2jax

    bass2jax.install_neuronx_cc_hook()
    partition_name = (nc.partition_id_tensor.name
                      if nc.partition_id_tensor else None)
    in_names, out_names, out_avals = [], [], []
    for alloc in nc.m.functions[0].allocations:
        if not isinstance(alloc, mybir.MemoryLocationSet):
            continue
        name = alloc.memorylocations[0].name
        if alloc.kind == "ExternalInput":
            if name != partition_name:
                in_names.append(name)
        elif alloc.kind == "ExternalOutput":
            out_names.append(name)
            out_avals.append(jax.core.ShapedArray(
                tuple(alloc.tensor_shape), mybir.dt.np(alloc.dtype)))
    n_params = len(in_names)
    all_in_names = list(in_names) + list(out_names)
    if partition_name is not None:
        all_in_names.append(partition_name)

    def _body(*args):
        operands = list(args)
        if partition_name is not None:
            operands.append(bass2jax.partition_id_tensor())
        outs = bass2jax._bass_exec_p.bind(
            *operands,
            out_avals=tuple(out_avals),
            in_names=tuple(all_in_names),
            out_names=tuple(out_names),
            lowering_input_output_aliases=(),
            sim_require_finite=True,
            sim_require_nnan=True,
            nc=nc,
        )
        return tuple(outs)

    devices = jax.devices()[:n_cores]
    mesh = Mesh(np.asarray(devices), ("core",))
    n_outs = len(out_names)
    in_specs = (PartitionSpec("core"),) * (n_params + n_outs)
    out_specs = (PartitionSpec("core"),) * n_outs
    fn = jax.jit(shard_map(_body, mesh=mesh, in_specs=in_specs,
                           out_specs=out_specs, check_rep=False),
                 keep_unused=True)
    sharding = NamedSharding(mesh, PartitionSpec("core"))

    def prepare(in_maps):
        concat_in = [
            np.concatenate([np.asarray(in_maps[c][n])
                            for c in range(n_cores)], axis=0)
            for n in in_names
        ]
        concat_zero = [
            np.zeros((n_cores * a.shape[0],) + tuple(a.shape[1:]), a.dtype)
            for a in out_avals
        ]
        return [jax.device_put(a, sharding) for a in concat_in + concat_zero]

    return fn, prepare, out_names


def _get_runner(key=(N_CORE, 1024, 512, 1, L0_MODE)):
    if key not in _RUNNER_CACHE:
        n_core, g_cols, f_cols, repeat, l0_mode = key
        nc = build_program(n_core, g_cols, f_cols, repeat, l0_mode=l0_mode)
        _RUNNER_CACHE[key] = make_runner(nc)
    return _RUNNER_CACHE[key]


def _get_runner_tiles(key=(N_CORE, 512, 1)):
    if key not in _RUNNER_CACHE:
        n_core, f_cols, repeat = key
        nc = build_program_tiles(n_core, f_cols, repeat)
        _RUNNER_CACHE[key] = make_runner(nc)
    return _RUNNER_CACHE[key]


def kernel(x, W0, b0, Wh, bh, Wo, bo):
    import jax
    x = np.asarray(x, np.float32)
    assert x.shape == (N_FULL, 3), x.shape
    if KERNEL_MODE == "tiles":
        consts = format_inputs_tiles(W0, b0, Wh, bh, Wo, bo)
        fn, prepare, out_names = _get_runner_tiles()
        in_maps = []
        for c in range(N_CORES):
            m = {"xt": host_pack_x_tiles(x[c * N_CORE:(c + 1) * N_CORE])}
            m.update(consts)
            in_maps.append(m)
        args = prepare(in_maps)
        outs = fn(*args)
        jax.block_until_ready(outs)
        y_raw = np.asarray(outs[out_names.index("y_raw")])
        y_raw = y_raw.reshape(N_CORES, 8, L32 * 4)
        y = np.concatenate(
            [host_unpack_y_tiles(y_raw[c]) for c in range(N_CORES)], axis=0)
        return np.ascontiguousarray(y.astype(np.float32))
    consts = format_inputs(W0, b0, Wh, bh, Wo, bo)
    fn, prepare, out_names = _get_runner()
    args = prepare(make_in_maps(x, consts))
    outs = fn(*args)
    jax.block_until_ready(outs)
    y = np.asarray(outs[out_names.index("y")])
    return np.ascontiguousarray(y.reshape(N_FULL, 1).astype(np.float32))
